# revision 95
# baseline (speedup 1.0000x reference)
"""Trainium2 Bass kernel for one CLIP transformer layer (pre-LN causal
attention + GELU FFN), data-parallel over batch across 8 NeuronCores.

Per core, one batch element, feature-major ("transposed") [d, s] layout so
matmul contractions run over the partition dim.

  host:  x.T in f32r (residual), bf16 x.T, and the LN1 column sums
         sum(x)/sum(x^2) precomputed; weights pre-transposed with LN
         gamma/beta + 1/sqrt(dh) folds; biases in per-tile [P, k] layouts
  LN:    rstd via ScalarE Sqrt (+eps in the activation bias) and the fast
         custom-DVE reciprocal on [1,s] rows, K=1 matmul broadcast, bf16
         whole-tensor apply with free-dim-broadcast scale rows; LN2 stats
         interleaved into the projection loop (f32r ones-matmuls)
  attn:  per head-pair, scores^T via two K=64 matmuls sharing the PE array
         (tile_position row packing), additive causal mask on diagonal
         blocks (one paired DVE op on [128,2,128]), exp on ScalarE over
         hb-merged [128,2,w] tiles (amortizes the 352-cyc ACT overhead),
         ragged AV with an appended ones column collecting softmax row
         sums; outputs evacuated UNNORMALIZED, row sums gathered one-per-
         partition and reciprocated in a single batched Newton pass (plain
         DVE ops; both custom reciprocal + [1,N] row chains are traps),
         then scaled via K=H one-hot-selector matmul broadcasts
  proj:  bf16, weight-stationary q-paired matmuls, weights/residual
         prefetched on the sync queue mid-attention
  FFN:   bf16 (fp8's 3-bit mantissa fails the 2e-2 gate here and on V,
         measured), q-paired matmuls, GELU fused into FFN1 evacuation,
         FFN2 + bias + residual in one DVE scalar_tensor_tensor
  all activation table sets (sqrt/exp/gelu) prefetched off-critical-path;
  x/wv loads split across the SP and Activation DGE queues
"""
import math
from contextlib import ExitStack

import numpy as np

import concourse.bass as bass
import concourse.mybir as mybir
import concourse.tile as tile
from concourse import bacc
from concourse.bass_utils import run_bass_kernel_spmd

B, S, D, H, FF = 8, 1024, 1024, 16, 4096
DH = D // H
EPS = 1e-5
P = 128
QC = 512                 # q-chunk width == one fp32 PSUM bank
NEG = -1e10              # additive causal mask value

f32 = mybir.dt.float32
f32r = mybir.dt.float32r
bf16 = mybir.dt.bfloat16
fp8 = mybir.dt.float8e4
i32 = mybir.dt.int32
ALU = mybir.AluOpType
ACTF = mybir.ActivationFunctionType
DR = mybir.MatmulPerfMode.DoubleRow

FP8_QKV = False          # h1T/wqk in fp8e4 + DoubleRow for Q/K
FP8_V = False            # V matmuls in fp8e4 DoubleRow (h1 copy + wv fp8)
SCALE_Q = 256.0 if FP8_QKV else 1.0   # pow2 prescale of wq
SCALE_K = 32.0 if FP8_QKV else 1.0
SCALE_V = 32.0 if (FP8_QKV or FP8_V) else 1.0
WARM_V = 16              # warm-up matmuls before the V phase
WARM_FFN = 24            # warm-up matmuls during LN2 finalize

TRACE = False            # set by test.py for profiled runs
LAST_RESULTS = None      # BassKernelResults of the most recent run

MAGIC = 0x5F3759DF


class _Pool:
    """Tile pool with explicit close() so SBUF is reclaimed mid-kernel."""

    def __init__(self, tc, **kw):
        self._cm = tc.tile_pool(**kw)
        self.pool = self._cm.__enter__()

    def tile(self, *a, **kw):
        if "name" not in kw:
            kw["name"] = kw.get("tag") or "t"
        return self.pool.tile(*a, **kw)

    def close(self):
        self._cm.__exit__(None, None, None)


def _ln_finalize(nc, tc, rp, sx, sxx, d, s, ones_1p, rs_sb, m2_sb,
                 warm=0, warm_sb=None):
    """From SBUF column-sum rows [1, s] (sum x, sum x^2) produce broadcast
    bf16 tiles rs_sb = d*rstd, m2_sb = Sx*rstd, each [P, s]. Row
    temporaries come from the caller's pool rp."""
    nq = s // QC
    with tc.tile_pool(name="lnbc", bufs=1, space="PSUM") as bcp:
        # sd = sqrt(d*Sxx - Sx^2 + d^2 eps); r = 1/sd
        # (4 row tiles rotate through 2 buffers)
        sx2 = rp.tile([1, s], f32, tag="row", bufs=2)
        nc.vector.tensor_mul(sx2, sx, sx)
        a = rp.tile([1, s], f32, tag="row", bufs=2)
        nc.vector.scalar_tensor_tensor(a, sxx, float(d), sx2,
                                       op0=ALU.mult, op1=ALU.subtract)
        eps_sb = rp.tile([1, 1], f32, tag="eps")
        nc.vector.memset(eps_sb, float(d) * d * EPS)
        sd = rp.tile([1, s], f32, tag="row", bufs=2)
        nc.scalar.activation(sd, a, ACTF.Sqrt, bias=eps_sb)
        r = rp.tile([1, s], f32, tag="row", bufs=2)
        nc.vector.reciprocal_approx_fast(r, sd)
        rs_row = rp.tile([1, s], bf16, tag="rsr")
        nc.vector.tensor_scalar_mul(rs_row, r, float(d))
        m2_row = rp.tile([1, s], bf16, tag="m2r")
        nc.vector.tensor_mul(m2_row, sx, r)

        bc_rs = bcp.tile([P, s], f32, tag="bcrs")
        bc_m2 = bcp.tile([P, s], f32, tag="bcm2")
        for q in range(nq):
            sl = slice(q * QC, (q + 1) * QC)
            nc.tensor.matmul(bc_rs[:, sl], ones_1p, rs_row[0:1, sl],
                             start=True, stop=True)
            nc.tensor.matmul(bc_m2[:, sl], ones_1p, m2_row[0:1, sl],
                             start=True, stop=True)
        if warm and warm_sb is not None:
            wp = bcp.tile([DH, QC], f32, tag="warm", bufs=1)
            for _ in range(warm):
                nc.tensor.matmul(wp, warm_sb[:, 0:DH], warm_sb,
                                 start=True, stop=True)
        nc.vector.tensor_copy(rs_sb, bc_rs)
        nc.vector.tensor_copy(m2_sb, bc_m2)


def build_nc(s=S):
    """Build the per-core Bass program (SPMD; identical on all 8 cores)."""
    dc = D // P              # feature chunks
    nq = s // QC             # q chunks
    kts = s // P             # k tiles
    nhp = H // 2             # head pairs
    nft = FF // P            # FFN hidden tiles
    kpq = QC // P            # k-tiles per q-chunk

    dt_qkv = fp8 if FP8_QKV else bf16

    nc = bacc.Bacc()
    xbT = nc.declare_dram_parameter("xbT", [D, s], bf16, isOutput=False)
    sxh = nc.declare_dram_parameter("sxh", [1, s], f32, isOutput=False)
    sxxh = nc.declare_dram_parameter("sxxh", [1, s], f32, isOutput=False)
    xT = nc.declare_dram_parameter("xT", [D, s], f32r, isOutput=False)
    wqkT = nc.declare_dram_parameter("wqkT", [D, 2 * D], dt_qkv,
                                     isOutput=False)
    dt_v = fp8 if (FP8_QKV or FP8_V) else bf16
    wvT = nc.declare_dram_parameter("wvT", [D, D], dt_v, isOutput=False)
    woT = nc.declare_dram_parameter("woT", [D, D], bf16, isOutput=False)
    w1T = nc.declare_dram_parameter("w1T", [D, FF], bf16, isOutput=False)
    w2T = nc.declare_dram_parameter("w2T", [FF, D], bf16, isOutput=False)
    bqk = nc.declare_dram_parameter("bqk", [P, 2 * dc], f32, isOutput=False)
    bo = nc.declare_dram_parameter("bo", [P, dc], f32, isOutput=False)
    b1 = nc.declare_dram_parameter("b1", [P, nft], f32, isOutput=False)
    b2 = nc.declare_dram_parameter("b2", [P, dc], f32, isOutput=False)
    mk2 = nc.declare_dram_parameter("mk2", [P, 2 * P], f32, isOutput=False)
    sel = nc.declare_dram_parameter("sel", [H, H * DH], bf16, isOutput=False)
    onesd = nc.declare_dram_parameter("onesd", [P, P], f32r, isOutput=False)
    onesb = nc.declare_dram_parameter("onesb", [P, P], bf16, isOutput=False)
    outT = nc.declare_dram_parameter("outT", [D, s], f32, isOutput=True)

    def chunked(t):
        return t.rearrange("(c p) n -> p c n", p=P)

    with tile.TileContext(nc, pool_alloc_mode="queue") as tc:
        with tc.tile_pool(name="glob", bufs=1) as g:
            # ---- small constants (issued first; x chunks right behind) ----
            ones_p1 = g.tile([P, 1], bf16)
            nc.scalar.dma_start(out=ones_p1, in_=onesb[:, 0:1])
            ones_1p = g.tile([1, P], bf16)
            nc.scalar.dma_start(out=ones_1p, in_=onesb[0:1, :])
            ones_r1 = g.tile([P, 1], f32r)
            nc.scalar.dma_start(out=ones_r1, in_=onesd[:, 0:1])
            warm_sb = g.tile([P, QC], bf16)
            for i in range(QC // P):
                nc.scalar.dma_start(out=warm_sb[:, i * P:(i + 1) * P],
                                 in_=onesb[:, :])
            mask2 = g.tile([P, 2, P], f32)
            nc.scalar.dma_start(out=mask2,
                                in_=mk2.rearrange("p (b n) -> p b n", b=2))
            bqk_sb = g.tile([P, 2 * dc], f32)
            nc.scalar.dma_start(out=bqk_sb, in_=bqk[:, :])
            bo_sb = g.tile([P, dc], f32)
            nc.scalar.dma_start(out=bo_sb, in_=bo[:, :])
            b1_sb = g.tile([P, nft], f32)
            nc.scalar.dma_start(out=b1_sb, in_=b1[:, :])
            b2_sb = g.tile([P, dc], f32)
            nc.scalar.dma_start(out=b2_sb, in_=b2[:, :])

            # scratch target for activation-table prefetches; load the sqrt
            # set immediately (LN1 finalize needs it first)
            dum = g.tile([1, 1], f32)
            nc.scalar.activation(dum, ones_p1[0:1, 0:1], ACTF.Sqrt)

            # ---- long-lived pools, opened in reverse close order (LIFO) --
            xap = _Pool(tc, name="xattn", bufs=1)
            xattnT = xap.tile([P, dc, s], f32r, tag="xattnT")
            h2p = _Pool(tc, name="h2", bufs=1)
            h2T = h2p.tile([P, dc, s], bf16, tag="h2T")
            rsm2_p = _Pool(tc, name="rsm2", bufs=1)
            rs2_sb = rsm2_p.tile([P, s], bf16, tag="rs2")
            m22_sb = rsm2_p.tile([P, s], bf16, tag="m22")
            sx2_row = rsm2_p.tile([1, s], f32, tag="sx2r")
            sxx2_row = rsm2_p.tile([1, s], f32, tag="sxx2r")
            otp = _Pool(tc, name="ot", bufs=1)
            oT = otp.tile([P, nhp, s], bf16, tag="oT")
            rs_all = otp.tile([H, s], bf16, tag="rs_all")
            sel_sb = otp.tile([H, H * DH], bf16, tag="sel")
            nc.scalar.dma_start(out=sel_sb, in_=sel[:, :])
            wxp = _Pool(tc, name="wx", bufs=1)
            h1p = _Pool(tc, name="h1", bufs=1)
            h1T = h1p.tile([P, dc, s], dt_qkv, tag="h1T")
            h1T8 = (h1p.tile([P, dc, s], fp8, tag="h1T8")
                    if FP8_V and not FP8_QKV else h1T)
            vp = _Pool(tc, name="v", bufs=1)
            v_sb = vp.tile([P, kts, H, DH + 1], bf16, tag="v_sb")
            wvp = _Pool(tc, name="wv", bufs=1)
            wv_sb = wvp.tile([P, dc, D], dt_v, tag="wv")

            # ---------------- LN1 (stats from host bf16 x, x^2) ----------
            rsm_p = _Pool(tc, name="rsm1", bufs=1)
            rs_sb = rsm_p.tile([P, s], bf16, tag="rs")
            m2_sb = rsm_p.tile([P, s], bf16, tag="m2")

            xin = _Pool(tc, name="xin", bufs=1)
            hc = dc // 2
            xbh = [xin.tile([P, hc, s], bf16, tag=f"xb{h}") for h in range(2)]
            xb_ch = chunked(xbT)
            sx_row = rsm_p.tile([1, s], f32, tag="sxr")
            sxx_row = rsm_p.tile([1, s], f32, tag="sxxr")
            nc.sync.dma_start(out=sx_row, in_=sxh[:, :])
            nc.sync.dma_start(out=sxx_row, in_=sxxh[:, :])
            # V weights split across both queues, interleaved with x
            wv_ch = chunked(wvT)
            for c in range(dc):
                eng = nc.sync if c % 2 == 0 else nc.scalar
                eng.dma_start(out=xbh[c // hc][:, c % hc, :],
                              in_=xb_ch[:, c, :])
            for c in range(dc):
                eng = nc.sync if c % 2 == 0 else nc.scalar
                eng.dma_start(out=wv_sb[:, c, :], in_=wv_ch[:, c, :])
            _ln_finalize(nc, tc, rsm_p, sx_row, sxx_row, D, s, ones_1p,
                         rs_sb, m2_sb, warm=WARM_V, warm_sb=warm_sb)
            # apply: h1 = xb*rs - m2 (4-chunk passes; rs/m2 broadcast over c)
            for qu in range(4):
                csl = slice(qu * 2, qu * 2 + 2)
                tmp = xin.tile([P, 2, s], bf16, tag="app", bufs=1)
                nc.vector.tensor_mul(
                    tmp, xbh[qu // 2][:, (qu % 2) * 2:(qu % 2) * 2 + 2, :],
                    rs_sb.unsqueeze(1).broadcast_to([P, 2, s]))
                nc.vector.tensor_sub(
                    h1T[:, csl, :], tmp,
                    m2_sb.unsqueeze(1).broadcast_to([P, 2, s]))
            if FP8_V and not FP8_QKV:
                nc.vector.tensor_copy(h1T8, h1T)
            xin.close()
            rsm_p.close()

            # prefetch the exp table set during the V phase
            nc.scalar.activation(dum, bqk_sb[0:1, 0:1], ACTF.Exp)

            # ------------- V = h1 @ WvT (natural layout, + ones col) ------
            nc.vector.memset(v_sb[:, :, :, DH:DH + 1], 1.0)
            with tc.tile_pool(name="vps", bufs=3, space="PSUM") as vps:
                hh = QC // DH  # heads per v-chunk
                for st in range(kts):
                    for vc in range(D // QC):
                        pv = vps.tile([P, QC], f32, tag="pv")
                        if FP8_QKV or FP8_V:
                            for c2 in range(dc // 2):
                                nc.tensor.matmul(
                                    pv,
                                    h1T8[:, 2 * c2:2 * c2 + 2,
                                         st * P:(st + 1) * P],
                                    wv_sb[:, 2 * c2:2 * c2 + 2,
                                          vc * QC:(vc + 1) * QC],
                                    start=(c2 == 0), stop=(c2 == dc // 2 - 1),
                                    perf_mode=DR)
                        else:
                            for c in range(dc):
                                nc.tensor.matmul(
                                    pv, h1T[:, c, st * P:(st + 1) * P],
                                    wv_sb[:, c, vc * QC:(vc + 1) * QC],
                                    start=(c == 0), stop=(c == dc - 1))
                        nc.scalar.activation(
                            v_sb[:, st, vc * hh:(vc + 1) * hh, 0:DH],
                            pv.rearrange("p (h e) -> p h e", h=hh),
                            ACTF.Identity, scale=1.0 / SCALE_V)

            wvp.close()

            # ---------------- attention, per head pair ----------------
            wo_ch = chunked(woT)
            xT_ch = chunked(xT)
            wo_tiles, xr_tiles = [], []
            with tc.tile_pool(name="wqk", bufs=3) as wqkp, \
                 tc.tile_pool(name="qk", bufs=4) as qkp, \
                 tc.tile_pool(name="at", bufs=4) as atp, \
                 tc.tile_pool(name="nrm", bufs=3) as nrmp, \
                 tc.tile_pool(name="qps", bufs=1, space="PSUM") as qps, \
                 tc.tile_pool(name="sps", bufs=2, space="PSUM") as sps, \
                 tc.tile_pool(name="avn", bufs=1, space="PSUM") as avn:
                wqk_ch = chunked(wqkT)
                for hp in range(nhp):
                    if hp == 2:
                        # prefetch projection weights + residual chunks on
                        # the sync queue behind the early wqk loads
                        for ot in range(dc):
                            wt = wxp.tile([P, dc, P], bf16, tag="wo", bufs=3)
                            nc.sync.dma_start(
                                out=wt, in_=wo_ch[:, :, ot * P:(ot + 1) * P])
                            wo_tiles.append(wt)
                            xr = wxp.tile([P, s], f32r, tag="xr", bufs=2)
                            nc.sync.dma_start(out=xr, in_=xT_ch[:, ot, :])
                            xr_tiles.append(xr)
                    qt = qkp.tile([P, s], bf16, tag="qt")
                    kt = qkp.tile([P, s], bf16, tag="kt")
                    for which, dst, qsc in ((0, qt, 1.0 / SCALE_Q),
                                            (1, kt, 1.0 / SCALE_K)):
                        wt = wqkp.tile([P, dc, P], dt_qkv, tag="w")
                        o0 = which * D + hp * P
                        nc.sync.dma_start(out=wt, in_=wqk_ch[:, :, o0:o0 + P])
                        bcol = which * dc + hp
                        for q in range(nq):
                            sl = slice(q * QC, (q + 1) * QC)
                            pq = qps.tile([P, QC], f32, tag="pq")
                            if FP8_QKV:
                                for c2 in range(dc // 2):
                                    nc.tensor.matmul(
                                        pq, wt[:, 2 * c2:2 * c2 + 2, :],
                                        h1T[:, 2 * c2:2 * c2 + 2, sl],
                                        start=(c2 == 0),
                                        stop=(c2 == dc // 2 - 1),
                                        perf_mode=DR)
                            else:
                                for c in range(dc):
                                    nc.tensor.matmul(
                                        pq, wt[:, c, :], h1T[:, c, sl],
                                        start=(c == 0), stop=(c == dc - 1))
                            nc.scalar.activation(
                                dst[:, sl], pq, ACTF.Identity,
                                bias=bqk_sb[:, bcol:bcol + 1], scale=qsc)
                    for q in range(nq):
                        sl = slice(q * QC, (q + 1) * QC)
                        po = [avn.tile([DH + 1, QC], f32, tag="po",
                                       name="po", bufs=2) for _ in range(2)]
                        nkt = (q + 1) * kpq
                        for ki in range(nkt):
                            r = ki * P - q * QC
                            c0 = max(r, 0)
                            w = QC - c0
                            qsl = slice(q * QC + c0, (q + 1) * QC)
                            ps2 = sps.tile([P, 2, QC], f32, tag="ps2")
                            for hb in range(2):
                                hsl = slice(hb * DH, (hb + 1) * DH)
                                nc.tensor.matmul(
                                    ps2[:, hb, 0:w],
                                    kt[hsl, ki * P:(ki + 1) * P],
                                    qt[hsl, qsl], start=True, stop=True)
                            if r >= 0:
                                nc.vector.tensor_add(
                                    ps2[:, :, 0:P], ps2[:, :, 0:P], mask2)
                            at2 = atp.tile([P, 2, QC], bf16, tag="at2")
                            nc.scalar.activation(at2[:, :, 0:w],
                                                 ps2[:, :, 0:w], ACTF.Exp)
                            for hb in range(2):
                                nc.tensor.matmul(
                                    po[hb][:, c0:QC],
                                    v_sb[:, ki, 2 * hp + hb, :],
                                    at2[:, hb, 0:w],
                                    start=(ki == 0), stop=(ki == nkt - 1))
                        # evacuate UNNORMALIZED attention outputs (plus the
                        # rowsum row); gather row sums into rs_all (one
                        # partition per head) for one batched reciprocal
                        for hb in range(2):
                            ob = nrmp.tile([DH + 1, QC], bf16, tag="ob", bufs=2)
                            nc.vector.tensor_copy(ob, po[hb])
                            nc.sync.dma_start(
                                out=rs_all[2 * hp + hb:2 * hp + hb + 1, sl],
                                in_=ob[DH:DH + 1, :])
                            nc.sync.dma_start(
                                out=oT[hb * DH:(hb + 1) * DH, hp, sl],
                                in_=ob[0:DH, :])
                # ---- batched softmax normalization (all heads at once) ----
                # 1/rowsum via Newton on [H, s] (H lanes in parallel; the
                # custom fast-reciprocal op is broken off partition 0)
                rsf = nrmp.tile([H, s], f32, tag="nwf", bufs=1)
                nc.vector.tensor_copy(rsf, rs_all)
                yt = nrmp.tile([H, s], f32, tag="nwy", bufs=1)
                nc.vector.tensor_scalar(yt.bitcast(i32), rsf.bitcast(i32),
                                        0, None, ALU.bitwise_not)
                nc.vector.tensor_scalar(yt.bitcast(i32), yt.bitcast(i32),
                                        0x7EF311C4, None, ALU.add)
                tn = nrmp.tile([H, s], f32, tag="nwt", bufs=1)
                for _ in range(2):
                    nc.vector.tensor_mul(tn, rsf, yt)
                    nc.vector.tensor_scalar(tn, tn, -1.0, 2.0,
                                            ALU.mult, ALU.add)
                    nc.vector.tensor_mul(yt, yt, tn)
                yr = nrmp.tile([H, s], bf16, tag="nwr", bufs=1)
                nc.vector.tensor_copy(yr, yt)
                # broadcast each head's scale over its 64 features (one
                # K=H matmul per head pair via a 0/1 selector) and scale
                # oT in place
                for hp in range(nhp):
                    for q in range(nq):
                        sl = slice(q * QC, (q + 1) * QC)
                        pbf = avn.tile([P, QC], f32, tag="pbf", bufs=1)
                        nc.tensor.matmul(
                            pbf, sel_sb[:, hp * P:(hp + 1) * P],
                            yr[:, sl], start=True, stop=True)
                        nc.vector.tensor_mul(oT[:, hp, sl], oT[:, hp, sl],
                                             pbf)
            vp.close()
            h1p.close()

            # prefetch the sqrt table set during the projection phase
            nc.scalar.activation(dum, bqk_sb[0:1, 0:1], ACTF.Sqrt)

            # -------- out-projection + residual, LN2 stats interleaved ----
            with tc.tile_pool(name="x2c", bufs=2) as x2p, \
                 tc.tile_pool(name="prs", bufs=2, space="PSUM") as prs, \
                 tc.tile_pool(name="ln2ps", bufs=1, space="PSUM") as lps2:
                ps_sx2 = lps2.tile([1, s], f32, tag="psx2")
                ps_sxx2 = lps2.tile([1, s], f32, tag="psxx2")
                for ot in range(dc):
                    pr2 = prs.tile([P, 2, QC], f32, tag="pr2")
                    for c in range(dc):
                        for q in range(nq):
                            sl = slice(q * QC, (q + 1) * QC)
                            nc.tensor.matmul(pr2[:, q, :], wo_tiles[ot][:, c, :],
                                             oT[:, c, sl],
                                             start=(c == 0), stop=(c == dc - 1))
                    nc.vector.scalar_tensor_tensor(
                        xattnT[:, ot, :].rearrange("p (q n) -> p q n", q=nq),
                        pr2, bo_sb[:, ot:ot + 1],
                        xr_tiles[ot].rearrange("p (q n) -> p q n", q=nq),
                        op0=ALU.add, op1=ALU.add)
                    xq2 = x2p.tile([P, s], bf16, tag="xq2")
                    nc.vector.tensor_mul(xq2, xattnT[:, ot, :],
                                         xattnT[:, ot, :])
                    for q in range(nq):
                        sl = slice(q * QC, (q + 1) * QC)
                        nc.tensor.matmul(ps_sx2[:, sl], ones_r1,
                                         xattnT[:, ot, sl],
                                         start=(ot == 0), stop=(ot == dc - 1))
                        nc.tensor.matmul(ps_sxx2[:, sl], ones_p1, xq2[:, sl],
                                         start=(ot == 0), stop=(ot == dc - 1))
                nc.vector.tensor_copy(sx2_row, ps_sx2)
                nc.vector.tensor_copy(sxx2_row, ps_sxx2)
            wxp.close()
            _ln_finalize(nc, tc, rsm2_p, sx2_row, sxx2_row, D, s, ones_1p,
                         rs2_sb, m22_sb, warm=WARM_FFN, warm_sb=warm_sb)
            # prefetch the gelu table set during the LN2 apply
            nc.scalar.activation(dum, bqk_sb[0:1, 0:1], ACTF.Gelu_apprx_tanh)
            otp.close()
            # LN2 apply (whole-tensor ops, broadcast scale rows)
            xb2f = _Pool(tc, name="xb2f", bufs=1)
            tmp2 = xb2f.tile([P, dc, s], bf16, tag="ap2")
            nc.vector.tensor_mul(tmp2, xattnT,
                                 rs2_sb.unsqueeze(1).broadcast_to([P, dc, s]))
            nc.vector.tensor_sub(h2T, tmp2,
                                 m22_sb.unsqueeze(1).broadcast_to([P, dc, s]))
            xb2f.close()
            rsm2_p.close()

            # ---------------- FFN ----------------
            with tc.tile_pool(name="aall", bufs=1) as aap, \
                 tc.tile_pool(name="w1", bufs=3) as w1p, \
                 tc.tile_pool(name="w2", bufs=3) as w2p, \
                 tc.tile_pool(name="yout", bufs=3) as youtp, \
                 tc.tile_pool(name="aps", bufs=2, space="PSUM") as aps, \
                 tc.tile_pool(name="yps", bufs=2, space="PSUM") as yps:
                a_all = aap.tile([P, nft, s], bf16, tag="a_all")
                w1_ch = chunked(w1T)
                w2_ch = chunked(w2T)
                for fc in range(nft):
                    wt = w1p.tile([P, dc, P], bf16, tag="w1")
                    nc.sync.dma_start(
                        out=wt, in_=w1_ch[:, :, fc * P:(fc + 1) * P])
                    pa2 = aps.tile([P, 2, QC], f32, tag="pa2")
                    for c in range(dc):
                        for q in range(nq):
                            sl = slice(q * QC, (q + 1) * QC)
                            nc.tensor.matmul(pa2[:, q, :], wt[:, c, :],
                                             h2T[:, c, sl],
                                             start=(c == 0), stop=(c == dc - 1))
                    nc.scalar.activation(
                        a_all[:, fc, :].rearrange("p (q n) -> p q n", q=nq),
                        pa2, ACTF.Gelu_apprx_tanh,
                        bias=b1_sb[:, fc:fc + 1])
                for do in range(dc):
                    wt = w2p.tile([P, nft, P], bf16, tag="w2")
                    nc.sync.dma_start(
                        out=wt, in_=w2_ch[:, :, do * P:(do + 1) * P])
                    py2 = yps.tile([P, 2, QC], f32, tag="py2")
                    for fi in range(nft):
                        for q in range(nq):
                            sl = slice(q * QC, (q + 1) * QC)
                            nc.tensor.matmul(py2[:, q, :], wt[:, fi, :],
                                             a_all[:, fi, sl],
                                             start=(fi == 0),
                                             stop=(fi == nft - 1))
                    y = youtp.tile([P, s], f32, tag="y")
                    nc.vector.scalar_tensor_tensor(
                        y.rearrange("p (q n) -> p q n", q=nq),
                        py2, b2_sb[:, do:do + 1],
                        xattnT[:, do, :].rearrange("p (q n) -> p q n", q=nq),
                        op0=ALU.add, op1=ALU.add)
                    nc.sync.dma_start(out=outT[do * P:(do + 1) * P, :], in_=y)
            h2p.close()
            xap.close()

    nc.compile()
    return nc


def prep_inputs(x, ln1_g, ln1_b, w_qkv, b_qkv, w_o, b_o, ln2_g, ln2_b,
                w1, b1, w2, b2, s=S):
    """Host-side preprocessing: LN gamma/beta folding, Q-scale folding,
    V-bias folding, fp8 pow2 prescales, transposes, bias layouts."""
    import ml_dtypes
    f = np.float32
    npb = ml_dtypes.bfloat16
    np8 = ml_dtypes.float8_e4m3
    x = np.asarray(x, f)
    ln1_g, ln1_b = np.asarray(ln1_g, f), np.asarray(ln1_b, f)
    ln2_g, ln2_b = np.asarray(ln2_g, f), np.asarray(ln2_b, f)
    w_qkv, b_qkv = np.asarray(w_qkv, f), np.asarray(b_qkv, f)
    w_o, b_o = np.asarray(w_o, f), np.asarray(b_o, f)
    w1, b1 = np.asarray(w1, f), np.asarray(b1, f)
    w2, b2 = np.asarray(w2, f), np.asarray(b2, f)

    wqkv_e = w_qkv * ln1_g[None, :]
    bqkv_e = b_qkv + w_qkv @ ln1_b
    sc = f(1.0 / math.sqrt(DH))
    wq = wqkv_e[0:D] * sc
    bq = bqkv_e[0:D] * sc
    wk, bk = wqkv_e[D:2 * D], bqkv_e[D:2 * D]
    wv, bv = wqkv_e[2 * D:], bqkv_e[2 * D:]

    dcn = D // P

    def to8(a, scale):
        return np.clip(a * f(scale), -240.0, 240.0).astype(np8)

    if FP8_QKV:
        wqk_h = np.concatenate([to8(wq, SCALE_Q), to8(wk, SCALE_K)], 0)
    else:
        wqk_h = np.concatenate([wq, wk], 0).astype(npb)
    if FP8_QKV or FP8_V:
        wv_h = to8(wv, SCALE_V)
    else:
        wv_h = (wv * f(SCALE_V)).astype(npb)
    tri = np.where(np.arange(P)[:, None] > np.arange(P)[None, :],
                   f(NEG), f(0.0))
    common = {
        "mk2": np.ascontiguousarray(np.concatenate([tri, tri], 1)),
        # sel[k, hp*128+m] = 1 iff k == 2*hp + (m >= 64): one K=H matmul
        # broadcasts head-pair hp's two softmax scales onto 128 partitions
        "sel": np.ascontiguousarray(
            (np.arange(H)[:, None]
             == (2 * (np.arange(H * DH)[None, :] // P)
                 + (np.arange(H * DH)[None, :] % P) // DH)).astype(npb)),
        "wqkT": np.ascontiguousarray(wqk_h.T),
        "wvT": np.ascontiguousarray(wv_h.T),
        "woT": np.ascontiguousarray(w_o.T).astype(npb),
        "w1T": np.ascontiguousarray((w1 * ln2_g[None, :]).T).astype(npb),
        "w2T": np.ascontiguousarray(w2.T).astype(npb),
        "bqk": np.ascontiguousarray(
            np.concatenate([bq, bk]).reshape(2 * dcn, P).T),
        "bo": np.ascontiguousarray((b_o + w_o @ bv).reshape(dcn, P).T),
        "b1": np.ascontiguousarray(
            (b1 + w1 @ ln2_b).reshape(FF // P, P).T),
        "b2": np.ascontiguousarray(b2.reshape(dcn, P).T),
        "onesd": np.ones((P, P), f),
        "onesb": np.ones((P, P), npb),
    }
    in_maps = []
    for b in range(x.shape[0]):
        xt = np.ascontiguousarray(x[b, :s].T)
        m = dict(common)
        m["xT"] = xt
        m["xbT"] = xt.astype(npb)
        m["sxh"] = xt.sum(0, dtype=f)[None, :]
        m["sxxh"] = (xt.astype(np.float64) ** 2).sum(0).astype(f)[None, :]
        in_maps.append(m)
    return in_maps


_NC_CACHE = {}


def kernel(**inputs) -> np.ndarray:
    global LAST_RESULTS
    if S not in _NC_CACHE:
        _NC_CACHE[S] = build_nc(S)
    nc = _NC_CACHE[S]
    in_maps = prep_inputs(**inputs)
    res = run_bass_kernel_spmd(nc, in_maps, core_ids=list(range(B)),
                               trace=TRACE)
    LAST_RESULTS = res
    out = np.stack([res.results[b]["outT"].T for b in range(B)])
    return np.ascontiguousarray(out.astype(np.float32))
